# revision 1
# baseline (speedup 1.0000x reference)
"""Trainium2 Bass kernel for nn_Decoder (LSTM decoder + attention + lm_head).

Sharding: data-parallel over batch (64 -> 8 cores x 8). Each core runs the
full pipeline for its batch shard locally; no collectives.

Per-core pipeline (one NEFF), feature-major recurrence:
  A) XGT[m] = W_ih[m-chunk] @ X.T for all steps (64 bf16 matmuls with
     stationary weight chunks; gates land feature-major in 16 chunks)
  B) 63 sequential LSTM cell steps, all feature-major:
     gatesT chunk[m] = XGT[m][:, t*8:+8] (id128 init matmul)
                       + sum_k W_hh.T[k][m] @ hT[k]      (N=8 moving)
     pointwise on [128,96]/[128,32] tiles (ACT sigmoid-only + DVE),
     h written directly feature-major (bf16 for matmuls, f32 for attention).
     tanh folded as 2*sigmoid(2x)-1 with the *2 pre-scaled into W_hh/W_in/
     W_out host-side (h is stored as h/2).
  C) Attention (f32): Q = W_in @ H.T; per batch element: scores via matmul
     with host-pretransposed encodings, masked exp via ACT bias, unnormalized
     ctx + denominator via matmuls, normalize with DVE reciprocal
  C2) Output projection (bf16) + tanh
  D) Vocab projection (bf16): logits = OUT @ W_lm.T + b_lm, streamed over
     32000 vocab in 512-wide banks, bias (bf16) added during PSUM eviction,
     fp16 output DMA
"""
import sys

sys.path.insert(0, "/opt/trn_rl_repo")

import numpy as np
import ml_dtypes

from concourse import bacc, bass, mybir
from concourse.tile import TileContext
from concourse.bass_utils import run_bass_kernel_spmd

f32 = mybir.dt.float32
fp16 = mybir.dt.float16
bf16 = mybir.dt.bfloat16
Act = mybir.ActivationFunctionType
Alu = mybir.AluOpType

NCORES = 8
T = 63            # decode steps (tgt_len - 1)
BL = 8            # batch per core
TOK = T * BL      # 504 tokens per core
TOKP = 512        # padded
SRC = 128
HID = 512
ENC = 512
INP = 512
V = 32000
GATES = 4 * HID   # 2048
NBANK = (V + 511) // 512  # 63 vocab banks (last = 256 wide)

# torch gate order i,f,g,o -> pipeline order f,i,g,o
PERM = np.concatenate([np.arange(512, 1024), np.arange(0, 512),
                       np.arange(1024, 1536), np.arange(1536, 2048)])

_BF = ml_dtypes.bfloat16


def _build(niter: int = 1, phases: str = "ABCD", dbg: bool = False,
           small_out: bool = False) -> "bacc.Bacc":
    nc = bacc.Bacc("TRN2", target_bir_lowering=False)

    xt_d = nc.dram_tensor("xt", [INP, TOKP], bf16, kind="ExternalInput")
    wih_d = nc.dram_tensor("wih", [INP, GATES], bf16, kind="ExternalInput")
    whh_d = nc.dram_tensor("whh", [HID, GATES], bf16, kind="ExternalInput")
    h0t_d = nc.dram_tensor("h0t", [128, 32], bf16, kind="ExternalInput")
    c0_d = nc.dram_tensor("c0", [128, 32], f32, kind="ExternalInput")
    enc_d = nc.dram_tensor("encf", [BL * SRC, ENC], f32, kind="ExternalInput")
    enct_d = nc.dram_tensor("enctf", [BL * ENC, SRC], f32, kind="ExternalInput")
    mbt_d = nc.dram_tensor("mbt", [SRC, BL], f32, kind="ExternalInput")
    win_d = nc.dram_tensor("wint", [HID, ENC], mybir.dt.float32r, kind="ExternalInput")
    wout_d = nc.dram_tensor("woutt", [ENC + HID, HID], bf16, kind="ExternalInput")
    wlm_d = nc.dram_tensor("wlmt", [HID, V], bf16, kind="ExternalInput")
    bbc_d = nc.dram_tensor("bbc", [128, V], bf16, kind="ExternalInput")
    id128_d = nc.dram_tensor("id128b", [128, 128], bf16, kind="ExternalInput")
    if small_out:
        out_d = nc.dram_tensor("logits", [128, 512], fp16, kind="ExternalOutput")
    else:
        out_d = nc.dram_tensor("logits", [TOKP, V], fp16, kind="ExternalOutput")

    with TileContext(nc) as tc:
        for i in range(niter):
            if i:
                tc.strict_bb_all_engine_barrier()
            _emit_iter(nc, tc, xt_d, wih_d, whh_d, h0t_d, c0_d, enc_d, enct_d,
                       mbt_d, win_d, wout_d, wlm_d, bbc_d, id128_d, out_d,
                       phases=phases, dbg=dbg, small_out=small_out)
    nc.compile()
    return nc


def _emit_iter(nc, tc, xt_d, wih_d, whh_d, h0t_d, c0_d, enc_d, enct_d, mbt_d,
               win_d, wout_d, wlm_d, bbc_d, id128_d, out_d,
               phases: str = "ABCD", dbg: bool = False, small_out: bool = False):
    MM = nc.tensor.matmul

    def dump(dst_row, tiles, width=TOKP):
        # debug: copy tiles (any dtype) as fp16 into logits[dst_row:+128, i*width..]
        with tc.tile_pool(name="dbg", bufs=2) as dp:
            for i, tl in enumerate(tiles):
                s = dp.tile([128, width], fp16, tag="d", name="dbgt")
                nc.vector.tensor_copy(s[:, 0:width], tl[:, 0:width])
                nc.sync.dma_start(
                    out=out_d[dst_row:dst_row + 128, i * width:(i + 1) * width],
                    in_=s[:, 0:width])

    with (
        tc.tile_pool(name="const", bufs=1) as cp,
        tc.tile_pool(name="store", bufs=1) as stp,
    ):
        # ---- resident tiles ----
        whh = [cp.tile([128, GATES], bf16, tag=f"whh{k}", name=f"whh{k}") for k in range(4)]
        h0t = cp.tile([128, 32], bf16, tag="h0t", name="h0t")
        nc.sync.dma_start(out=h0t[:], in_=h0t_d[:])
        c0 = cp.tile([128, 32], f32, tag="c0", name="c0")
        nc.sync.dma_start(out=c0[:], in_=c0_d[:])
        id128 = cp.tile([128, 128], bf16, tag="id128", name="id128")
        nc.sync.dma_start(out=id128[:], in_=id128_d[:])
        # attention-phase constants: tiles allocated now, DMAs emitted after
        # phase A so they stream during the (DMA-idle) LSTM recurrence
        enc_sb = [cp.tile([SRC, ENC], f32, tag=f"enc{b}", name=f"enc{b}") for b in range(BL)]
        enctb = [cp.tile([128, 4 * SRC], f32, tag=f"enct{b}", name=f"enct{b}") for b in range(BL)]
        enct_sb = [[enctb[b][:, k * SRC:(k + 1) * SRC] for k in range(4)] for b in range(BL)]
        mbt = cp.tile([SRC, BL], f32, tag="mbt", name="mbt")
        wint = [cp.tile([128, ENC], mybir.dt.float32r, tag=f"wint{k}", name=f"wint{k}") for k in range(4)]
        woutt = [cp.tile([128, HID], bf16, tag=f"woutt{k}", name=f"woutt{k}") for k in range(8)]

        def emit_attn_const_dmas():
            for b in range(BL):
                nc.sync.dma_start(out=enc_sb[b][:], in_=enc_d[b * SRC:(b + 1) * SRC, :])
            for b in range(BL):
                nc.sync.dma_start(
                    out=enctb[b][:].rearrange("p (k s) -> p k s", k=4),
                    in_=enct_d[b * ENC:(b + 1) * ENC, :].rearrange("(k p) s -> p k s", k=4))
            nc.sync.dma_start(out=mbt[:], in_=mbt_d[:])
            for k in range(4):
                nc.sync.dma_start(out=wint[k][:], in_=win_d[k * 128:(k + 1) * 128, :])
            for k in range(8):
                nc.sync.dma_start(out=woutt[k][:], in_=wout_d[k * 128:(k + 1) * 128, :])
        ones_s = cp.tile([128, 1], f32, tag="ones_s", name="ones_s")
        nc.any.memset(ones_s[:], 1.0)
        ones_1 = cp.tile([1, 128], f32, tag="ones_1", name="ones_1")
        nc.any.memset(ones_1[:], 1.0)

        # ---- accumulating stores ----
        xgtB = stp.tile([128, TOKP * 16], bf16, tag="xgtB", name="xgtB")
        htbB = stp.tile([128, 4 * TOKP], bf16, tag="htbB", name="htbB")
        htb = [htbB[:, k * TOKP:(k + 1) * TOKP] for k in range(4)]
        qtf = [stp.tile([128, TOKP], f32, tag=f"qtf{m}", name=f"qtf{m}") for m in range(4)]
        # b-major f32r h store, written directly by the recurrence:
        # col layout k*TOKP + b*T + j  (j = step)
        htfbmB = stp.tile([128, 4 * TOKP], mybir.dt.float32r, tag="htfbmB", name="htfbmB")
        htfbm = [htfbmB[:, k * TOKP:(k + 1) * TOKP] for k in range(4)]
        ctxt = [stp.tile([128, TOKP], bf16, tag=f"ctxt{k}", name=f"ctxt{k}") for k in range(4)]
        outt = [stp.tile([128, TOKP], bf16, tag=f"outt{m}", name=f"outt{m}") for m in range(4)]
        for m in range(4):
            # zero the 8 pad-token columns so phase D can use full 128-row tiles
            nc.any.memset(outt[m][:, TOK:TOKP], 0.0)

        # ================= Phase A: XGT = (W_ih @ X.T) =================
        # XGT chunk m (128 gate-features) x TOKP tokens, bf16
        if "A" not in phases:
            return
        with (
            tc.tile_pool(name="pa_sb", bufs=1) as pa_sb,
            tc.tile_pool(name="pa_ps", bufs=2, space="PSUM") as pa_ps,
        ):
            xts = [pa_sb.tile([128, TOKP], bf16, tag=f"xt{k}", name=f"xt{k}") for k in range(4)]
            for k in range(4):
                nc.sync.dma_start(out=xts[k][:], in_=xt_d[k * 128:(k + 1) * 128, :])
            wih = [pa_sb.tile([128, GATES], bf16, tag=f"wih{k}", name=f"wih{k}") for k in range(4)]
            for k in range(4):
                nc.sync.dma_start(out=wih[k][:], in_=wih_d[k * 128:(k + 1) * 128, :])
            for k in range(4):
                nc.sync.dma_start(out=whh[k][:], in_=whh_d[k * 128:(k + 1) * 128, :])
            for m in range(16):
                ps = pa_ps.tile([128, TOKP], f32, tag="pa", name="pa")
                for k in range(4):
                    MM(ps[:], wih[k][:, m * 128:(m + 1) * 128], xts[k][:],
                       start=(k == 0), stop=(k == 3))
                xgv = xgtB[:].rearrange("p (t c) -> p t c", c=128)[:, :, m * 8:(m + 1) * 8]
                psv = ps[:].rearrange("p (t b) -> p t b", b=8)
                nc.scalar.copy(xgv, psv)

        emit_attn_const_dmas()


        # ================= Phase B: LSTM recurrence (feature-major) ======
        if "B" not in phases:
            return
        with (
            tc.tile_pool(name="pb_tmp", bufs=3) as pb_tmp,
            tc.tile_pool(name="pb_c", bufs=2) as pb_c,
            tc.tile_pool(name="pb_g", bufs=3, space="PSUM") as pb_g,
        ):
            c_prev = c0
            for t in range(T):
                # gatesT in one PSUM tile: chunk m at cols m*8..m*8+8
                # layout: f = cols 0:32, i = 32:64, g = 64:96, o = 96:128
                G = pb_g.tile([128, 128], f32, tag="G", name="G")
                # one start=True matmul covering the whole bank: later
                # per-region matmuls accumulate without clearing has_written
                MM(G[:], id128[:], xgtB[:, t * 128:(t + 1) * 128],
                   start=True, stop=False)
                for m in range(16):
                    for k in range(4):
                        rhs = (h0t[:, k * 8:(k + 1) * 8] if t == 0 else
                               htbB[:, k * TOKP + (t - 1) * 8:
                                    k * TOKP + t * 8])
                        MM(G[:, m * 8:(m + 1) * 8],
                           whh[k][:, m * 128:(m + 1) * 128], rhs,
                           start=False, stop=(k == 3))
                # pointwise, all feature-major. ACT ops all Sigmoid
                # (tanh(x) = 2*sigmoid(2x)-1; the *2 of h is pre-folded into
                # W_hh/W_in/W_out host-side, so h is stored as h/2).
                sall = pb_tmp.tile([128, 128], f32, tag="sall", name="sall")
                nc.scalar.activation(sall[:, 0:32], G[:, 0:32], Act.Sigmoid)
                c1 = pb_tmp.tile([128, 32], f32, tag="c1", name="c1")
                nc.vector.tensor_mul(c1[:], sall[:, 0:32], c_prev[:])
                nc.scalar.activation(sall[:, 32:96], G[:, 32:96], Act.Sigmoid)
                nc.scalar.activation(sall[:, 96:128], G[:, 96:128], Act.Sigmoid)
                # up = (sig(2g) - 0.5) * sig(i)  == sig(i)*tanh(g)/2
                up = pb_tmp.tile([128, 32], f32, tag="up", name="up")
                nc.vector.scalar_tensor_tensor(
                    up[:], sall[:, 64:96], 0.5, sall[:, 32:64],
                    op0=Alu.subtract, op1=Alu.mult)
                c_new = pb_c.tile([128, 32], f32, tag="c", name="c")
                nc.vector.scalar_tensor_tensor(
                    c_new[:], up[:], 2.0, c1[:],
                    op0=Alu.mult, op1=Alu.add)
                tc2 = pb_tmp.tile([128, 32], f32, tag="tc2", name="tc2")
                nc.scalar.activation(tc2[:], c_new[:], Act.Sigmoid, scale=2.0)
                # h/2 = (sig(2c) - 0.5) * sig(o), stored feature-major
                tv = tc2[:].rearrange("p (k c) -> p k c", k=4)
                sv = sall[:, 96:128].rearrange("p (k c) -> p k c", k=4)
                hbv = htbB[:].rearrange("p (k c) -> p k c", k=4)[:, :, t * 8:(t + 1) * 8]
                hfv = (htfbmB[:].rearrange("p (k r) -> p k r", k=4)[:, :, 0:TOK]
                       .rearrange("p k (b j) -> p k b j", b=BL)[:, :, :, t])
                nc.vector.scalar_tensor_tensor(
                    hbv, tv, 0.5, sv, op0=Alu.subtract, op1=Alu.mult)
                nc.vector.scalar_tensor_tensor(
                    hfv, tv, 0.5, sv, op0=Alu.subtract, op1=Alu.mult)
                c_prev = c_new

        if dbg:
            dump(128, [htfbm[k] for k in range(4)], width=TOK)
        # ================= Phase C: attention (f32) =================
        if "C" not in phases:
            return
        with tc.tile_pool(name="pq_ps", bufs=2, space="PSUM") as pq_ps:
            for m in range(4):
                ps = pq_ps.tile([128, TOK], f32, tag="q", name="q")
                for k in range(4):
                    MM(ps[:], wint[k][:, m * 128:(m + 1) * 128],
                       htfbm[k][:, 0:TOK], start=(k == 0), stop=(k == 3))
                nc.scalar.copy(qtf[m][:, 0:TOK], ps[:])

        TB = T * BL  # 504, b-major stage layout: col = b*T + j
        with (
            tc.tile_pool(name="pc_sb", bufs=1) as pc_sb,
            tc.tile_pool(name="pc_s", bufs=1, space="PSUM") as pc_s,
            tc.tile_pool(name="pc_d", bufs=1, space="PSUM") as pc_d,
            tc.tile_pool(name="pc_b", bufs=1, space="PSUM") as pc_b,
            tc.tile_pool(name="pc_c", bufs=1, space="PSUM") as pc_c,
        ):
            pss = pc_s.tile([SRC, TB], f32, tag="scores", name="scores")
            for b in range(BL):
                for k in range(4):
                    MM(pss[:, b * T:(b + 1) * T], enct_sb[b][k],
                       qtf[k][:, b * T:(b + 1) * T], start=(k == 0), stop=(k == 3))
            e_all = pc_sb.tile([SRC, TB], f32, tag="e_all", name="e_all")
            for b in range(BL):
                nc.scalar.activation(e_all[:, b * T:(b + 1) * T],
                                     pss[:, b * T:(b + 1) * T], Act.Exp,
                                     bias=mbt[:, b:b + 1])
            psd = pc_d.tile([1, TB], f32, tag="denom", name="denom")
            MM(psd[:], ones_s[:], e_all[:], start=True, stop=True)
            rec = pc_sb.tile([1, TB], f32, tag="rec", name="rec")
            nc.vector.reciprocal(rec[:], psd[:])
            psb = pc_b.tile([128, TB], f32, tag="recb_ps", name="recb_ps")
            MM(psb[:], ones_1[:], rec[:], start=True, stop=True)
            recb = pc_sb.tile([128, TB], f32, tag="recb", name="recb")
            nc.scalar.copy(recb[:], psb[:])
            recv = recb[:].rearrange("p (b j) -> p b j", b=BL)
            for k in range(4):
                psc = pc_c.tile([128, TB], f32, tag=f"ctx{k}", name=f"ctx{k}")
                for b in range(BL):
                    MM(psc[:, b * T:(b + 1) * T],
                       enc_sb[b][:, k * 128:(k + 1) * 128],
                       e_all[:, b * T:(b + 1) * T], start=True, stop=True)
                # normalize + scatter b-major -> token-major in one strided mul
                ctxv = ctxt[k][:, 0:TOK].rearrange("p (j b) -> p b j", b=BL)
                pscv = psc[:].rearrange("p (b j) -> p b j", b=BL)
                nc.vector.tensor_mul(ctxv, pscv, recv)

        if dbg:
            dump(256, qtf, width=TOK)
            dump(384, ctxt, width=TOK)
        # ================= Phase C2: out-projection + tanh =================
        with tc.tile_pool(name="po_ps", bufs=2, space="PSUM") as po_ps:
            for m in range(4):
                ps = po_ps.tile([128, TOK], f32, tag="o", name="o")
                for k in range(8):
                    rhs = ctxt[k] if k < 4 else htb[k - 4]
                    MM(ps[:], woutt[k][:, m * 128:(m + 1) * 128],
                       rhs[:, 0:TOK], start=(k == 0), stop=(k == 7))
                nc.scalar.activation(outt[m][:, 0:TOK], ps[:], Act.Tanh)


        # ================= Phase D: vocab projection =================
        if "D" not in phases:
            return
        with (
            tc.tile_pool(name="pd_w", bufs=8) as pd_w,
            tc.tile_pool(name="pd_b", bufs=4) as pd_b,
            tc.tile_pool(name="pd_st", bufs=4) as pd_st,
            tc.tile_pool(name="pd_ps", bufs=6, space="PSUM") as pd_ps,
        ):
            for nb in range(NBANK):
                n0 = nb * 512
                nw = min(512, V - n0)
                # single strided DMA for all 4 k-chunks of this vocab bank
                wl4 = pd_w.tile([128, 4 * 512], bf16, tag="wl4", name="wl4")
                nc.sync.dma_start(
                    out=wl4[:].rearrange("p (k n) -> p k n", k=4)[:, :, 0:nw],
                    in_=wlm_d[:, n0:n0 + nw].rearrange("(k p) n -> p k n", k=4))
                bb = pd_b.tile([128, 512], bf16, tag="bb", name="bb")
                nc.sync.dma_start(out=bb[:, 0:nw], in_=bbc_d[:, n0:n0 + nw])
                st4 = pd_st.tile([128, 4 * 512], fp16, tag="st4", name="st4")
                for mt in range(4):
                    m0 = mt * 128
                    ps = pd_ps.tile([128, 512], f32, tag="v", name="v")
                    for k in range(4):
                        MM(ps[:, 0:nw], outt[k][:, m0:m0 + 128],
                           wl4[:, k * 512:k * 512 + nw],
                           start=(k == 0), stop=(k == 3))
                    nc.vector.tensor_add(st4[:, mt * 512:mt * 512 + nw],
                                         ps[:, 0:nw], bb[:, 0:nw])
                if small_out:
                    nc.sync.dma_start(out=out_d[:, 0:nw], in_=st4[:, 0:nw])
                else:
                    # one DMA writes all 512 (padded) token rows of this bank;
                    # SBUF-side AP keeps the partition dim first
                    nc.sync.dma_start(
                        out=out_d[:, n0:n0 + nw].rearrange("(m p) n -> p m n", m=4),
                        in_=st4[:].rearrange("p (m n) -> p m n", m=4)[:, :, 0:nw])


def _prep_in_maps(inputs: dict) -> list[dict]:
    targets = np.asarray(inputs["targets"])
    mask = np.asarray(inputs["attention_mask"])
    enc = np.asarray(inputs["encodings"], dtype=np.float32)
    h = np.asarray(inputs["h"], dtype=np.float32)
    c = np.asarray(inputs["c"], dtype=np.float32)
    emb = np.asarray(inputs["emb"], dtype=np.float32)
    W_ih = np.asarray(inputs["W_ih"], dtype=np.float32)
    W_hh = np.asarray(inputs["W_hh"], dtype=np.float32)
    W_in = np.asarray(inputs["W_in"], dtype=np.float32)
    W_out = np.asarray(inputs["W_out"], dtype=np.float32)
    W_lm = np.asarray(inputs["W_lm"], dtype=np.float32)
    b_lm = np.asarray(inputs["b_lm"], dtype=np.float32)

    x_seq = emb[targets[:-1]]                      # (63, 64, 512)
    # gate-g rows doubled so the single on-chip sigmoid yields sig(2g)
    # (tanh(g) = 2*sig(2g) - 1); h is stored as h/2 on-chip, so W_hh is
    # doubled once more for every gate.
    wih_g2 = W_ih[PERM].copy()
    wih_g2[1024:1536] *= 2.0
    wih_p = wih_g2.T.astype(_BF).copy()            # (512, 2048)
    whh_g2 = (2.0 * W_hh)[PERM].copy()
    whh_g2[1024:1536] *= 2.0
    whh_p = whh_g2.T.astype(_BF).copy()
    wint = (2.0 * W_in).T.astype(np.float32).copy()  # (512, 512)
    W_out2 = W_out.copy()
    W_out2[:, ENC:] *= 2.0
    woutt = W_out2.T.astype(_BF).copy()            # (1024, 512)
    wlmt = W_lm.T.astype(_BF).copy()               # (512, 32000)
    bbc = np.broadcast_to(b_lm, (128, V)).astype(_BF).copy()
    id128b = np.eye(128, dtype=_BF)

    def fmajor(a, dtype):
        # (BL, 512) batch-major -> feature-major [128, 4*8] chunk layout
        at = np.ascontiguousarray(a.T).reshape(4, 128, BL)
        return np.concatenate([at[k] for k in range(4)], axis=1).astype(dtype)

    in_maps = []
    for cidx in range(NCORES):
        sl = slice(cidx * BL, (cidx + 1) * BL)
        xt = np.zeros((INP, TOKP), np.float32)
        xt[:, :TOK] = x_seq[:, sl, :].reshape(TOK, INP).T
        h0t = fmajor(h[sl] * 0.5, _BF)                            # h0/2
        c0t = fmajor(c[sl], np.float32)
        encc = enc[:, sl, :]                                      # (128, 8, 512)
        encf = np.ascontiguousarray(encc.transpose(1, 0, 2)).reshape(BL * SRC, ENC)
        enctf = np.ascontiguousarray(encc.transpose(1, 2, 0)).reshape(BL * ENC, SRC)
        mbt = np.where(mask[:, sl], np.float32(-1e30), np.float32(0.0)).astype(np.float32)
        in_maps.append({
            "xt": xt.astype(_BF),
            "wih": wih_p, "whh": whh_p,
            "h0t": h0t,
            "c0": c0t,
            "encf": encf.astype(np.float32),
            "enctf": enctf.astype(np.float32),
            "mbt": mbt,
            "wint": wint, "woutt": woutt, "wlmt": wlmt, "bbc": bbc,
            "id128b": id128b,
        })
    return in_maps


def _assemble(results) -> np.ndarray:
    out = np.empty((T, 64, V), np.float32)
    for cidx in range(NCORES):
        lg = results[cidx]["logits"][:TOK].astype(np.float32).reshape(T, BL, V)
        out[:, cidx * BL:(cidx + 1) * BL, :] = lg
    return out


_CACHE: dict = {}


def kernel(**inputs) -> np.ndarray:
    if "nc" not in _CACHE:
        _CACHE["nc"] = _build(niter=1)
    in_maps = _prep_in_maps(inputs)
    res = run_bass_kernel_spmd(_CACHE["nc"], in_maps, core_ids=list(range(NCORES)))
    return _assemble(res.results)

